# revision 19
# baseline (speedup 1.0000x reference)
"""Adaptive Huber/MSE/L1 loss on 8 TRN2 NeuronCores (Bass/Tile), v8 (= best-measured v6b config).

Reference math (per sample, N = 4,096,000 elements):
    e   = pred - true
    L2  = mean(e^2);  L1 = mean(|e|)
    huber = (S2 - SR) * 0.5 / N     (S2 = sum e^2, SR = sum relu(|e|-5)^2)
    use_l2 = (L2 <= 1) | (L2 < L1^2)
    loss = mean_over_batch(where(use_l2, L2, huber))

Sharding: data-parallel, sample i -> core i. The host interleaves pred
and true at DMA-tile granularity into ONE [128, 64000] f32 DRAM tensor
per core (one dma_start / one completion semaphore per tile). Each core
emits a [1,48] f32 row of partial sums in S2 | SR | S1 column regions;
the host sums the regions and finishes the branch math during unshard.

Measured facts driving this layout (v4-v7 HW traces, this session):
  - Stream runs at ~430 GB/s = ~98% of the 435 GB/s SBUF-AXI fabric
    ceiling; 32.77 MB arrives over [5.2us, 86.5us]. That is the floor.
  - DVE keeps a self-contained per-tile chain: sub (1.12ns/col, f32
    in), u16 |e| mask (0.34), max->m (0.34). v7 tried replacing the
    mask with ACT's Abs pass feeding DVE's max - the cross-engine
    round trip HOL-blocked DVE every pair and the stream collapsed to
    ~330 GB/s. Keep DVE dependencies DVE-local.
  - ACT passes cost ~0.975ns/col + 0.28us ACCUM_READ + ~0.18us gap per
    instruction; per-tile e^2+m^2 accums (5.18us per 2000-col tile)
    exceed the 4.84us arrival budget, so main-body Square passes are
    PAIRED across two tiles sharing one buffer (one pass + one read).
  - m^2 split: DVE mult + PE row-sum chunks on {4,7,10,13} and the
    tail, ACT Square pairs on the rest. Shifting the late m^2 mults to
    ACT instead (v8 experiment) made ACT the global straggler and lost
    7us; this balance measured best (98.6us).
  - The ~250-instruction per-semaphore NEFF epilogue (~6us) and ~5us
    preamble are compiler-generated and fixed (queue-trim fails NRT).
    tensor_tensor_reduce and GpSimd tensor ops die at compile/NRT;
    DVE tensor_scalar rejects abs_max. All probed - don't reintroduce.
  - Pool ring slots chain WAR deps in ALLOCATION order: allocate
    work buffers in usage order or early tiles wait on later readers
    (measured: stream collapse at 80us).

Tail tiles shrink 1500/800/500/200; the 200-col last tile runs a short
DVE-local chain, ACT reduces the S1 PSUM chain via Identity+accum in
parallel, then one fp32 partition-collapse matmul and a 192 B output
DMA finish the kernel.
"""

import numpy as np

import concourse.bass as bass
import concourse.bacc as bacc
import concourse.mybir as mybir
from concourse.tile import TileContext
from concourse.bass_utils import run_bass_kernel_spmd

P = 128
COLS = 32000  # 160*160*160 / 128
DELTA = 5.0
N_CORES = 8
N_ELEM = float(P * COLS)
CHUNK = 500  # PE reduction column-chunk (PSUM bank limit 512 f32)
NF = 48  # fin columns: [0,20)=S2, [20,40)=SR, [40,48)=S1

F32 = mybir.dt.float32
U16 = mybir.dt.uint16
BF16 = mybir.dt.bfloat16
ALU = mybir.AluOpType
ACTF = mybir.ActivationFunctionType
AX = mybir.AxisListType

TILES = [2500, 2500] + [2000] * 12 + [1500, 800, 500, 200]
LAST = len(TILES) - 1  # t17: fully-DVE final tile
# e^2 ACT Square pass pairs (both tiles' |e| share one buffer)
E2_PAIRS = [(0, 1), (2, 3), (4, 5), (6, 7), (8, 9), (10, 11), (12, 13)]
E2_SINGLE = {14, 15, 16}
# m^2: ACT pairs for non-DVE main tiles; DVE tiles and the tail mult
# on V + PE chunks into the d2 chain (closes t16)
M2_PAIRS = [(0, 1), (2, 3), (5, 6), (8, 9), (11, 12)]
M2_DVE = {4, 7, 10, 13, 14, 15, 16}
M2_SINGLE = set()

# fin columns
S2_COL = {p: i for i, p in enumerate(E2_PAIRS)}  # pair -> col
S2_COL.update({14: 7, 15: 8, 16: 9, 17: 10})
SR_COL = {p: 20 + i for i, p in enumerate(M2_PAIRS)}
SR_D2 = 25
SR_T17 = 26
S1_COL = 40
S1_T17 = 41


def build():
    assert sum(TILES) == COLS
    e2_partner = {}
    for a, b in E2_PAIRS:
        e2_partner[a] = (a, b)
        e2_partner[b] = (a, b)
    m2_partner = {}
    for a, b in M2_PAIRS:
        m2_partner[a] = (a, b)
        m2_partner[b] = (a, b)

    mm_s1 = sum(
        (w + CHUNK - 1) // CHUNK for t, w in enumerate(TILES) if t != LAST
    )
    mm_d2 = sum((TILES[t] + CHUNK - 1) // CHUNK for t in M2_DVE)

    nc = bacc.Bacc(
        "TRN2",
        target_bir_lowering=False,
        debug=False,
        enable_asserts=False,
        num_devices=N_CORES,
    )
    x_ext = nc.dram_tensor("x", [P, 2 * COLS], F32, kind="ExternalInput")
    out_ext = nc.dram_tensor("out", [1, NF], F32, kind="ExternalOutput")

    with TileContext(nc) as tc:
        with (
            tc.tile_pool(name="iob", bufs=2) as iob_pool,
            tc.tile_pool(name="iom", bufs=5) as iom_pool,
            tc.tile_pool(name="work", bufs=3) as work_pool,
            tc.tile_pool(name="acc", bufs=1) as acc_pool,
            tc.tile_pool(name="psum", bufs=1, space="PSUM") as psum_pool,
        ):
            fin = acc_pool.tile([P, NF], F32)
            fin2 = acc_pool.tile([1, NF], F32)
            scr = acc_pool.tile([P, 5000], BF16)  # ACT pass output sink
            ones_bf = acc_pool.tile([P, 1], BF16)
            ones_f = acc_pool.tile([P, 1], F32)
            nc.vector.memset(ones_bf[:], 1.0)
            nc.vector.memset(ones_f[:], 1.0)
            nc.vector.memset(fin[:], 0.0)
            psum_s1 = psum_pool.tile([1, CHUNK], F32)  # S1 chain t0..16
            psum_d2 = psum_pool.tile([1, CHUNK], F32)  # m^2 chain, M2_DVE
            ps2 = psum_pool.tile([1, NF], F32)

            io_tiles = []
            col = 0
            for t, w in enumerate(TILES):
                pool = iob_pool if t < 2 else iom_pool
                xt = pool.tile([P, 2 * w], F32, tag="xb" if t < 2 else "xm")
                nc.sync.dma_start(out=xt[:], in_=x_ext[:, 2 * col : 2 * col + 2 * w])
                io_tiles.append(xt)
                col += w
            assert col == COLS

            # pair buffers: both members' |e| / m land in one tile so one
            # ACT Square pass + one ACCUM_READ covers the pair.
            # Allocated in USAGE order (see docstring).
            e_bufs = {}  # tile -> (buf, offset)
            m_bufs = {}

            mm_i = 0
            mmd_i = 0
            for t, w in enumerate(TILES):
                if t not in e_bufs:
                    if t in e2_partner:
                        a, b = e2_partner[t]
                        buf = work_pool.tile(
                            [P, TILES[a] + TILES[b]], BF16, tag="e",
                            name=f"ep{a}_{b}",
                        )
                        e_bufs[a] = (buf, 0)
                        e_bufs[b] = (buf, TILES[a])
                    else:
                        buf = work_pool.tile(
                            [P, w], BF16, tag="e", name=f"es{t}"
                        )
                        e_bufs[t] = (buf, 0)
                if t not in m_bufs:
                    if t in m2_partner:
                        a, b = m2_partner[t]
                        buf = work_pool.tile(
                            [P, TILES[a] + TILES[b]], BF16, tag="m",
                            name=f"mp{a}_{b}",
                        )
                        m_bufs[a] = (buf, 0)
                        m_bufs[b] = (buf, TILES[a])
                    else:
                        buf = work_pool.tile(
                            [P, w], BF16, tag="m", name=f"ms{t}"
                        )
                        m_bufs[t] = (buf, 0)
                xt = io_tiles[t]
                ebuf, eo = e_bufs[t]
                mbuf, mo = m_bufs[t]
                e = ebuf[:, eo : eo + w]
                m = mbuf[:, mo : mo + w]
                eu = ebuf.bitcast(U16)[:, eo : eo + w]
                # e = a - b (bf16 out: unbiased rounding, ~1e-5 rel err
                # on the final loss, far under the 2e-2 gate)
                nc.vector.tensor_tensor(e, xt[:, 0:w], xt[:, w : 2 * w], ALU.subtract)
                # |e| in place via u16 mask (2x 16-bit mode)
                nc.vector.tensor_scalar(eu, eu, 0x7FFF, None, ALU.bitwise_and)
                # m = max(|e|,5) - 5 == relu(|e|-5)
                nc.vector.tensor_scalar(m, e, DELTA, -DELTA, ALU.max, ALU.add)
                # S1: PE ones^T @ |e| chunks, one chain t0..16
                if t != LAST:
                    nch = (w + CHUNK - 1) // CHUNK
                    for c in range(nch):
                        cw = min(CHUNK, w - c * CHUNK)
                        nc.tensor.matmul(
                            psum_s1[0:1, 0:cw], ones_bf[:, 0:1],
                            ebuf[:, eo + c * CHUNK : eo + c * CHUNK + cw],
                            start=(mm_i == 0), stop=(mm_i == mm_s1 - 1),
                        )
                        mm_i += 1
                else:
                    nc.vector.tensor_reduce(
                        fin[:, S1_T17 : S1_T17 + 1], e, axis=AX.X,
                        op=ALU.add, apply_absolute_value=True,
                    )
                # m^2
                if t in M2_DVE:
                    nc.vector.tensor_tensor(m, m, m, ALU.mult)
                    nch = (w + CHUNK - 1) // CHUNK
                    for c in range(nch):
                        cw = min(CHUNK, w - c * CHUNK)
                        nc.tensor.matmul(
                            psum_d2[0:1, 0:cw], ones_bf[:, 0:1],
                            mbuf[:, mo + c * CHUNK : mo + c * CHUNK + cw],
                            start=(mmd_i == 0), stop=(mmd_i == mm_d2 - 1),
                        )
                        mmd_i += 1
                elif t in m2_partner:
                    pa, pb = m2_partner[t]
                    if t == pb:  # pair complete -> one ACT pass
                        pw = TILES[pa] + TILES[pb]
                        nc.scalar.activation(
                            scr[:, 0:pw], m_bufs[pa][0][:, 0:pw], ACTF.Square,
                            accum_out=fin[:, SR_COL[(pa, pb)] : SR_COL[(pa, pb)] + 1],
                        )
                elif t in M2_SINGLE:
                    nc.scalar.activation(
                        scr[:, 0:w], m, ACTF.Square,
                        accum_out=fin[:, SR_COL[t] : SR_COL[t] + 1],
                    )
                else:  # t == LAST
                    nc.vector.tensor_tensor(m, m, m, ALU.mult)
                    nc.vector.tensor_reduce(
                        fin[:, SR_T17 : SR_T17 + 1], m, axis=AX.X, op=ALU.add
                    )
                # e^2
                if t in e2_partner:
                    pa, pb = e2_partner[t]
                    if t == pb:
                        pw = TILES[pa] + TILES[pb]
                        nc.scalar.activation(
                            scr[:, 0:pw], e_bufs[pa][0][:, 0:pw], ACTF.Square,
                            accum_out=fin[:, S2_COL[(pa, pb)] : S2_COL[(pa, pb)] + 1],
                        )
                elif t in E2_SINGLE:
                    nc.scalar.activation(
                        scr[:, 0:w], e, ACTF.Square,
                        accum_out=fin[:, S2_COL[t] : S2_COL[t] + 1],
                    )
                else:  # t == LAST: square in place on DVE, plain reduce
                    nc.vector.tensor_tensor(e, e, e, ALU.mult)
                    nc.vector.tensor_reduce(
                        fin[:, S2_COL[t] : S2_COL[t] + 1], e, axis=AX.X, op=ALU.add
                    )
            assert mm_i == mm_s1 and mmd_i == mm_d2

            # [1,500] PSUM chain reduces on ACT (Identity + accumulator),
            # emitted after the last tile so they follow e^2(t16) on the
            # in-order ACT queue; both chains closed at t16's PE chunks
            nc.scalar.activation(
                scr[0:1, 0:CHUNK], psum_s1[0:1, :], ACTF.Identity,
                accum_out=fin[0:1, S1_COL : S1_COL + 1],
            )
            nc.scalar.activation(
                scr[0:1, 0:CHUNK], psum_d2[0:1, :], ACTF.Identity,
                accum_out=fin[0:1, SR_D2 : SR_D2 + 1],
            )

            # partition-collapse so the output is one 192 B DMA packet
            nc.tensor.matmul(ps2[0:1, 0:NF], ones_f[:, 0:1], fin[:, 0:NF],
                             start=True, stop=True)
            nc.vector.tensor_scalar(fin2[:], ps2[0:1, 0:NF], 1.0, None, ALU.mult)
            nc.sync.dma_start(out=out_ext[:, :], in_=fin2[:])

    nc.compile()
    return nc


_NC_CACHE = {}


def _get_nc():
    if "nc" not in _NC_CACHE:
        _NC_CACHE["nc"] = build()
    return _NC_CACHE["nc"]


def _pack(a: np.ndarray, b: np.ndarray) -> np.ndarray:
    """Interleave pred/true at DMA-tile granularity: one [P, 2*COLS]
    tensor per core, tile t occupying cols [2*off, 2*off+2*w) with the
    pred block first and the true block second."""
    x = np.empty((N_CORES, P, 2 * COLS), dtype=np.float32)
    off = 0
    for w in TILES:
        x[:, :, 2 * off : 2 * off + w] = a[:, :, off : off + w]
        x[:, :, 2 * off + w : 2 * off + 2 * w] = b[:, :, off : off + w]
        off += w
    return x


def kernel(y_pred_logits: np.ndarray, y_true: np.ndarray, _trace=False) -> np.ndarray:
    nc = _get_nc()
    a = np.ascontiguousarray(y_pred_logits, dtype=np.float32).reshape(N_CORES, P, COLS)
    b = np.ascontiguousarray(y_true, dtype=np.float32).reshape(N_CORES, P, COLS)
    x = _pack(a, b)
    in_maps = [{"x": x[i]} for i in range(N_CORES)]
    # the fleet occasionally reports a transient NRT_EXEC_UNIT_UNRECOVERABLE
    # from a prior aborted run; it clears on retry
    last_err = None
    for attempt in range(3):
        try:
            r = run_bass_kernel_spmd(
                nc, in_maps, core_ids=list(range(N_CORES)), trace=_trace
            )
            break
        except Exception as exc:  # noqa: BLE001
            import traceback

            print(f"[kernel] attempt {attempt} failed: {exc!r}")
            traceback.print_exc()
            last_err = exc
            import time

            time.sleep(10.0)
    else:
        raise last_err
    per_sample = np.empty(N_CORES, dtype=np.float64)
    for i in range(N_CORES):
        row = np.asarray(r.results[i]["out"], dtype=np.float64).ravel()
        s2 = row[0:20].sum()
        sr = row[20:40].sum()
        s1 = row[40:48].sum()
        l2 = s2 / N_ELEM
        l1 = s1 / N_ELEM
        huber = 0.5 * (s2 - sr) / N_ELEM
        per_sample[i] = l2 if (l2 <= 1.0 or l2 < l1 * l1) else huber
    out = np.float32(per_sample.mean()).reshape(())
    if _trace:
        return out, r
    return out


# revision 20
# speedup vs baseline: 1.1555x; 1.1555x over previous
"""Adaptive Huber/MSE/L1 loss on 8 TRN2 NeuronCores (Bass/Tile), v8 (= best-measured v6b config).

Reference math (per sample, N = 4,096,000 elements):
    e   = pred - true
    L2  = mean(e^2);  L1 = mean(|e|)
    huber = (S2 - SR) * 0.5 / N     (S2 = sum e^2, SR = sum relu(|e|-5)^2)
    use_l2 = (L2 <= 1) | (L2 < L1^2)
    loss = mean_over_batch(where(use_l2, L2, huber))

Sharding: data-parallel, sample i -> core i. The host interleaves pred
and true at DMA-tile granularity into ONE [128, 64000] f32 DRAM tensor
per core (one dma_start / one completion semaphore per tile). Each core
emits a [1,48] f32 row of partial sums in S2 | SR | S1 column regions;
the host sums the regions and finishes the branch math during unshard.

Measured facts driving this layout (v4-v7 HW traces, this session):
  - Stream runs at ~430 GB/s = ~98% of the 435 GB/s SBUF-AXI fabric
    ceiling; 32.77 MB arrives over [5.2us, 86.5us]. That is the floor.
  - DVE keeps a self-contained per-tile chain: sub (1.12ns/col, f32
    in), u16 |e| mask (0.34), max->m (0.34). v7 tried replacing the
    mask with ACT's Abs pass feeding DVE's max - the cross-engine
    round trip HOL-blocked DVE every pair and the stream collapsed to
    ~330 GB/s. Keep DVE dependencies DVE-local.
  - ACT passes cost ~0.975ns/col + 0.28us ACCUM_READ + ~0.18us gap per
    instruction; per-tile e^2+m^2 accums (5.18us per 2000-col tile)
    exceed the 4.84us arrival budget, so main-body Square passes are
    PAIRED across two tiles sharing one buffer (one pass + one read).
  - m^2 split: DVE mult + PE row-sum chunks on {4,7,10,13} and the
    tail, ACT Square pairs on the rest. Shifting the late m^2 mults to
    ACT instead (v8 experiment) made ACT the global straggler and lost
    7us; this balance measured best (98.6us).
  - The ~250-instruction per-semaphore NEFF epilogue (~6us) and ~5us
    preamble are compiler-generated and fixed (queue-trim fails NRT).
    tensor_tensor_reduce and GpSimd tensor ops die at compile/NRT;
    DVE tensor_scalar rejects abs_max. All probed - don't reintroduce.
  - Pool ring slots chain WAR deps in ALLOCATION order: allocate
    work buffers in usage order or early tiles wait on later readers
    (measured: stream collapse at 80us).
  - Run-to-run fleet noise is large (identical NEFFs measured 98.6us
    and 114.4us; engine active times identical, stalls differ). The
    8-deep io ring (~39us of buffered stream) is insurance: transient
    compute lag must exceed it before the DMA stream backpressures.

Tail tiles shrink 1500/1000/800/500/200; the 200-col last tile runs a short
DVE-local chain, ACT reduces the S1 PSUM chain via Identity+accum in
parallel, then one fp32 partition-collapse matmul and a 192 B output
DMA finish the kernel.
"""

import numpy as np

import concourse.bass as bass
import concourse.bacc as bacc
import concourse.mybir as mybir
from concourse.tile import TileContext
from concourse.bass_utils import run_bass_kernel_spmd

P = 128
COLS = 32000  # 160*160*160 / 128
DELTA = 5.0
N_CORES = 8
N_ELEM = float(P * COLS)
CHUNK = 500  # PE reduction column-chunk (PSUM bank limit 512 f32)
NF = 48  # fin columns: [0,20)=S2, [20,40)=SR, [40,48)=S1

F32 = mybir.dt.float32
U16 = mybir.dt.uint16
BF16 = mybir.dt.bfloat16
ALU = mybir.AluOpType
ACTF = mybir.ActivationFunctionType
AX = mybir.AxisListType

TILES = [2000] * 14 + [1500, 1000, 800, 500, 200]
LAST = len(TILES) - 1  # t18: fully-DVE final tile
# e^2 ACT Square pass pairs (both tiles' |e| share one buffer)
E2_PAIRS = [(0, 1), (2, 3), (4, 5), (6, 7), (8, 9), (10, 11), (12, 13)]
E2_SINGLE = {14, 15, 16, 17}
# m^2: ACT pairs for non-DVE main tiles; DVE tiles and the tail mult
# on V + PE chunks into the d2 chain (closes t16)
M2_PAIRS = [(0, 1), (2, 3), (5, 6), (8, 9), (11, 12)]
M2_DVE = {4, 7, 10, 13, 14, 15, 16, 17}
M2_SINGLE = set()

# fin columns
S2_COL = {p: i for i, p in enumerate(E2_PAIRS)}  # pair -> col
S2_COL.update({14: 7, 15: 8, 16: 9, 17: 10, 18: 11})
SR_COL = {p: 20 + i for i, p in enumerate(M2_PAIRS)}
SR_D2 = 25
SR_T18 = 26
S1_COL = 40
S1_T18 = 41


def build():
    assert sum(TILES) == COLS
    e2_partner = {}
    for a, b in E2_PAIRS:
        e2_partner[a] = (a, b)
        e2_partner[b] = (a, b)
    m2_partner = {}
    for a, b in M2_PAIRS:
        m2_partner[a] = (a, b)
        m2_partner[b] = (a, b)

    mm_s1 = sum(
        (w + CHUNK - 1) // CHUNK for t, w in enumerate(TILES) if t != LAST
    )
    mm_d2 = sum((TILES[t] + CHUNK - 1) // CHUNK for t in M2_DVE)

    nc = bacc.Bacc(
        "TRN2",
        target_bir_lowering=False,
        debug=False,
        enable_asserts=False,
        num_devices=N_CORES,
    )
    x_ext = nc.dram_tensor("x", [P, 2 * COLS], F32, kind="ExternalInput")
    out_ext = nc.dram_tensor("out", [1, NF], F32, kind="ExternalOutput")

    with TileContext(nc) as tc:
        with (
            tc.tile_pool(name="io", bufs=8) as io_pool,
            tc.tile_pool(name="work", bufs=3) as work_pool,
            tc.tile_pool(name="acc", bufs=1) as acc_pool,
            tc.tile_pool(name="psum", bufs=1, space="PSUM") as psum_pool,
        ):
            fin = acc_pool.tile([P, NF], F32)
            fin2 = acc_pool.tile([1, NF], F32)
            scr = acc_pool.tile([P, 4000], BF16)  # ACT pass output sink
            ones_bf = acc_pool.tile([P, 1], BF16)
            ones_f = acc_pool.tile([P, 1], F32)
            nc.vector.memset(ones_bf[:], 1.0)
            nc.vector.memset(ones_f[:], 1.0)
            nc.vector.memset(fin[:], 0.0)
            psum_s1 = psum_pool.tile([1, CHUNK], F32)  # S1 chain t0..16
            psum_d2 = psum_pool.tile([1, CHUNK], F32)  # m^2 chain, M2_DVE
            ps2 = psum_pool.tile([1, NF], F32)

            io_tiles = []
            col = 0
            for t, w in enumerate(TILES):
                xt = io_pool.tile([P, 2 * w], F32, tag="x")
                nc.sync.dma_start(out=xt[:], in_=x_ext[:, 2 * col : 2 * col + 2 * w])
                io_tiles.append(xt)
                col += w
            assert col == COLS

            # pair buffers: both members' |e| / m land in one tile so one
            # ACT Square pass + one ACCUM_READ covers the pair.
            # Allocated in USAGE order (see docstring).
            e_bufs = {}  # tile -> (buf, offset)
            m_bufs = {}

            mm_i = 0
            mmd_i = 0
            for t, w in enumerate(TILES):
                if t not in e_bufs:
                    if t in e2_partner:
                        a, b = e2_partner[t]
                        buf = work_pool.tile(
                            [P, TILES[a] + TILES[b]], BF16, tag="e",
                            name=f"ep{a}_{b}",
                        )
                        e_bufs[a] = (buf, 0)
                        e_bufs[b] = (buf, TILES[a])
                    else:
                        buf = work_pool.tile(
                            [P, w], BF16, tag="e", name=f"es{t}"
                        )
                        e_bufs[t] = (buf, 0)
                if t not in m_bufs:
                    if t in m2_partner:
                        a, b = m2_partner[t]
                        buf = work_pool.tile(
                            [P, TILES[a] + TILES[b]], BF16, tag="m",
                            name=f"mp{a}_{b}",
                        )
                        m_bufs[a] = (buf, 0)
                        m_bufs[b] = (buf, TILES[a])
                    else:
                        buf = work_pool.tile(
                            [P, w], BF16, tag="m", name=f"ms{t}"
                        )
                        m_bufs[t] = (buf, 0)
                xt = io_tiles[t]
                ebuf, eo = e_bufs[t]
                mbuf, mo = m_bufs[t]
                e = ebuf[:, eo : eo + w]
                m = mbuf[:, mo : mo + w]
                eu = ebuf.bitcast(U16)[:, eo : eo + w]
                # e = a - b (bf16 out: unbiased rounding, ~1e-5 rel err
                # on the final loss, far under the 2e-2 gate)
                nc.vector.tensor_tensor(e, xt[:, 0:w], xt[:, w : 2 * w], ALU.subtract)
                # |e| in place via u16 mask (2x 16-bit mode)
                nc.vector.tensor_scalar(eu, eu, 0x7FFF, None, ALU.bitwise_and)
                # m = max(|e|,5) - 5 == relu(|e|-5)
                nc.vector.tensor_scalar(m, e, DELTA, -DELTA, ALU.max, ALU.add)
                # S1: PE ones^T @ |e| chunks, one chain t0..16
                if t != LAST:
                    nch = (w + CHUNK - 1) // CHUNK
                    for c in range(nch):
                        cw = min(CHUNK, w - c * CHUNK)
                        nc.tensor.matmul(
                            psum_s1[0:1, 0:cw], ones_bf[:, 0:1],
                            ebuf[:, eo + c * CHUNK : eo + c * CHUNK + cw],
                            start=(mm_i == 0), stop=(mm_i == mm_s1 - 1),
                        )
                        mm_i += 1
                else:
                    nc.vector.tensor_reduce(
                        fin[:, S1_T18 : S1_T18 + 1], e, axis=AX.X,
                        op=ALU.add, apply_absolute_value=True,
                    )
                # m^2
                if t in M2_DVE:
                    nc.vector.tensor_tensor(m, m, m, ALU.mult)
                    nch = (w + CHUNK - 1) // CHUNK
                    for c in range(nch):
                        cw = min(CHUNK, w - c * CHUNK)
                        nc.tensor.matmul(
                            psum_d2[0:1, 0:cw], ones_bf[:, 0:1],
                            mbuf[:, mo + c * CHUNK : mo + c * CHUNK + cw],
                            start=(mmd_i == 0), stop=(mmd_i == mm_d2 - 1),
                        )
                        mmd_i += 1
                elif t in m2_partner:
                    pa, pb = m2_partner[t]
                    if t == pb:  # pair complete -> one ACT pass
                        pw = TILES[pa] + TILES[pb]
                        nc.scalar.activation(
                            scr[:, 0:pw], m_bufs[pa][0][:, 0:pw], ACTF.Square,
                            accum_out=fin[:, SR_COL[(pa, pb)] : SR_COL[(pa, pb)] + 1],
                        )
                elif t in M2_SINGLE:
                    nc.scalar.activation(
                        scr[:, 0:w], m, ACTF.Square,
                        accum_out=fin[:, SR_COL[t] : SR_COL[t] + 1],
                    )
                else:  # t == LAST
                    nc.vector.tensor_tensor(m, m, m, ALU.mult)
                    nc.vector.tensor_reduce(
                        fin[:, SR_T18 : SR_T18 + 1], m, axis=AX.X, op=ALU.add
                    )
                # e^2
                if t in e2_partner:
                    pa, pb = e2_partner[t]
                    if t == pb:
                        pw = TILES[pa] + TILES[pb]
                        nc.scalar.activation(
                            scr[:, 0:pw], e_bufs[pa][0][:, 0:pw], ACTF.Square,
                            accum_out=fin[:, S2_COL[(pa, pb)] : S2_COL[(pa, pb)] + 1],
                        )
                elif t in E2_SINGLE:
                    nc.scalar.activation(
                        scr[:, 0:w], e, ACTF.Square,
                        accum_out=fin[:, S2_COL[t] : S2_COL[t] + 1],
                    )
                else:  # t == LAST: square in place on DVE, plain reduce
                    nc.vector.tensor_tensor(e, e, e, ALU.mult)
                    nc.vector.tensor_reduce(
                        fin[:, S2_COL[t] : S2_COL[t] + 1], e, axis=AX.X, op=ALU.add
                    )
            assert mm_i == mm_s1 and mmd_i == mm_d2

            # [1,500] PSUM chain reduces on ACT (Identity + accumulator),
            # emitted after the last tile so they follow e^2(t16) on the
            # in-order ACT queue; both chains closed at t16's PE chunks
            nc.scalar.activation(
                scr[0:1, 0:CHUNK], psum_s1[0:1, :], ACTF.Identity,
                accum_out=fin[0:1, S1_COL : S1_COL + 1],
            )
            nc.scalar.activation(
                scr[0:1, 0:CHUNK], psum_d2[0:1, :], ACTF.Identity,
                accum_out=fin[0:1, SR_D2 : SR_D2 + 1],
            )

            # partition-collapse so the output is one 192 B DMA packet
            nc.tensor.matmul(ps2[0:1, 0:NF], ones_f[:, 0:1], fin[:, 0:NF],
                             start=True, stop=True)
            nc.vector.tensor_scalar(fin2[:], ps2[0:1, 0:NF], 1.0, None, ALU.mult)
            nc.sync.dma_start(out=out_ext[:, :], in_=fin2[:])

    nc.compile()
    return nc


_NC_CACHE = {}


def _get_nc():
    if "nc" not in _NC_CACHE:
        _NC_CACHE["nc"] = build()
    return _NC_CACHE["nc"]


def _pack(a: np.ndarray, b: np.ndarray) -> np.ndarray:
    """Interleave pred/true at DMA-tile granularity: one [P, 2*COLS]
    tensor per core, tile t occupying cols [2*off, 2*off+2*w) with the
    pred block first and the true block second."""
    x = np.empty((N_CORES, P, 2 * COLS), dtype=np.float32)
    off = 0
    for w in TILES:
        x[:, :, 2 * off : 2 * off + w] = a[:, :, off : off + w]
        x[:, :, 2 * off + w : 2 * off + 2 * w] = b[:, :, off : off + w]
        off += w
    return x


def kernel(y_pred_logits: np.ndarray, y_true: np.ndarray, _trace=False) -> np.ndarray:
    nc = _get_nc()
    a = np.ascontiguousarray(y_pred_logits, dtype=np.float32).reshape(N_CORES, P, COLS)
    b = np.ascontiguousarray(y_true, dtype=np.float32).reshape(N_CORES, P, COLS)
    x = _pack(a, b)
    in_maps = [{"x": x[i]} for i in range(N_CORES)]
    # the fleet occasionally reports a transient NRT_EXEC_UNIT_UNRECOVERABLE
    # from a prior aborted run; it clears on retry
    last_err = None
    for attempt in range(3):
        try:
            r = run_bass_kernel_spmd(
                nc, in_maps, core_ids=list(range(N_CORES)), trace=_trace
            )
            break
        except Exception as exc:  # noqa: BLE001
            import traceback

            print(f"[kernel] attempt {attempt} failed: {exc!r}")
            traceback.print_exc()
            last_err = exc
            import time

            time.sleep(10.0)
    else:
        raise last_err
    per_sample = np.empty(N_CORES, dtype=np.float64)
    for i in range(N_CORES):
        row = np.asarray(r.results[i]["out"], dtype=np.float64).ravel()
        s2 = row[0:20].sum()
        sr = row[20:40].sum()
        s1 = row[40:48].sum()
        l2 = s2 / N_ELEM
        l1 = s1 / N_ELEM
        huber = 0.5 * (s2 - sr) / N_ELEM
        per_sample[i] = l2 if (l2 <= 1.0 or l2 < l1 * l1) else huber
    out = np.float32(per_sample.mean()).reshape(())
    if _trace:
        return out, r
    return out
